# revision 26
# baseline (speedup 1.0000x reference)
"""Haar DWT (512x512, levels=1) on 8 Trainium2 NeuronCores.

Input  x: [8, 64, 512, 512] f32  (plus the four Haar band matrices, which
are fixed/deterministic and therefore folded into the kernel math).
Output: (LL, LH, HL, HH), each [8, 64, 256, 256] f32.

Strategy: pure data parallel over the batch dim (core i handles x[i]).
All HBM traffic is fp16 (grading tolerance is 2e-2 rel; fp16 adds ~4e-4)
and the Haar /2 is folded into the host-side cast (x*0.5, exact).

The key layout trick: the host pre-deinterleaves even/odd image COLUMNS
(a pure permutation, folded into the same host-side cast/copy pass that
already exists for the fp16 conversion). With the two column phases
stored as separate contiguous halves, the horizontal butterfly becomes
`even_half +- odd_half` on unit-stride fp16 operands, and the vertical
butterfly pairs adjacent rows within a partition (gappy but unit-stride
inner dim). All six DVE ops per tile therefore run in the 2x perf mode
(2-byte dtype + innermost stride 1 + <=2 free AP dims), unlike the naive
in-order layout whose stride-2 horizontal pass is stuck at 1x. DVE busy
~146us, under the ~152us DMA roofline (64MB/core at the measured
~420GB/s/core aggregate cap), so no PE/ACT assist is needed.

DMA: per unit of 4 images, loads are 4x 512KB dma_starts with 4KB
descriptor runs on the sync queue (4KB is the measured packet sweet
spot; 8KB runs and multi-queue/fine-grained stores all measured slower
end-to-end) and the store is one merged 2MB dma_start on the scalar
queue (bands in one dram tensor, 4KB runs).

Hard-won scheduling facts (each measured as ~+30us when violated):
 - GpSimd must stay COMPLETELY idle -- any op or DMA trigger there
   starves the DMA descriptor path (Q7 cores back it).
 - fio bufs=3 is load-bearing; bufs=2 serializes the pipeline.
 - DVE ops with 3+ free AP dims drop out of 2x mode (1.5ns/elem).
Variants tried and measurably worse: fp8-e4m3 half-input with ACT
upcast (DVE becomes the pacer, +7us), PE-matmul vertical stage for a
subset of images (baseline-style hybrid, +24us), 8KB runs (+16us),
band-interleaved output layout with per-band store chunks (+30us).
"""

import numpy as np


def _ensure_concourse():
    try:
        import concourse.bass  # noqa: F401
    except ImportError:
        import sys

        for p in ("/opt/trn_rl_repo", "/root/.axon_site/_ro/trn_rl_repo"):
            if p not in sys.path:
                sys.path.append(p)
        import concourse.bass  # noqa: F401


N_CORES = 8
IMG = 512  # image height == width
BANDS = ("ll", "lh", "hl", "hh")
# band order inside the merged output tensor
BAND_IDX = {"ll": 0, "lh": 1, "hl": 2, "hh": 3}


def build_nc(n_images=64):
    """Build the single-core Bass program (SPMD: same program on all cores)."""
    _ensure_concourse()
    from concourse import bacc, mybir
    from concourse.tile import TileContext

    f16 = mybir.dt.float16
    # NOTE: keep enable_partition_id at its default (True). Building with
    # False removes a ~3.7 us preamble TENSOR_LOAD but the axon PJRT execute
    # path requires the trailing partition-id parameter and the NEFF faults
    # with NRT_EXEC_UNIT_UNRECOVERABLE without it.
    nc = bacc.Bacc("TRN2", target_bir_lowering=False, debug=False)

    # x layout (host-prepped): [img, g=32, eo=2, u=16, w=256] so that each
    # of the 128 partitions (c g) of a 4-image unit owns 16KB contiguous
    # DRAM: 16 consecutive rows' even-column half then odd-column half.
    x = nc.dram_tensor("x", [n_images, 32, 2, 16, 256], f16,
                       kind="ExternalInput")
    o = nc.dram_tensor("o", [4, n_images, IMG // 2, IMG // 2], f16,
                       kind="ExternalOutput")

    CI = 4          # images per unit
    FX = 2048 * CI  # free elems per partition of the input tile

    with TileContext(nc) as tc:
        with (
            tc.tile_pool(name="fio", bufs=3) as fio_pool,
            tc.tile_pool(name="fmid", bufs=3) as fmid_pool,
            tc.tile_pool(name="fws", bufs=3) as fws_pool,
        ):
            def emit_unit(i0):
                xv = x[i0 : i0 + CI].rearrange("c g eo u w -> (c g) (eo u w)")
                xt = fio_pool.tile([128, FX], f16, tag="x")
                # 4KB descriptor runs (measured best per-packet rate; 16KB
                # packets degrade ~20% under load, 2KB measured 20.5 B/ns
                # vs 4KB's 23-25)
                for k in range(FX // 2048):
                    nc.sync.dma_start(
                        out=xt[:, k * 2048 : (k + 1) * 2048],
                        in_=xv[:, k * 2048 : (k + 1) * 2048],
                    )

                # horizontal butterfly: even half +- odd half, all unit
                # stride -> 2x mode. cs = col sums, cd = col difs.
                xtv = xt[:].rearrange("p (eo m) -> p eo m", eo=2)
                cs = fmid_pool.tile([128, FX // 2], f16, tag="cs")
                cd = fmid_pool.tile([128, FX // 2], f16, tag="cd")
                nc.vector.tensor_add(cs[:], xtv[:, 0], xtv[:, 1])
                nc.vector.tensor_sub(cd[:], xtv[:, 0], xtv[:, 1])

                # vertical butterfly: adjacent row pairs within a partition
                # (inner dim w=256 unit stride -> still 2x mode), written
                # into the four band blocks of one merged store tile.
                ws = fws_pool.tile([128, FX], f16, tag="ws")
                wv = ws[:].rearrange("p (b j w) -> p b j w", b=4, w=256)
                c4 = cs[:].rearrange("p (j eo w) -> p j eo w", eo=2, w=256)
                d4 = cd[:].rearrange("p (j eo w) -> p j eo w", eo=2, w=256)
                nc.vector.tensor_add(wv[:, 0], c4[:, :, 0], c4[:, :, 1])  # LL
                nc.vector.tensor_add(wv[:, 1], d4[:, :, 0], d4[:, :, 1])  # LH
                nc.vector.tensor_sub(wv[:, 2], c4[:, :, 0], c4[:, :, 1])  # HL
                nc.vector.tensor_sub(wv[:, 3], d4[:, :, 0], d4[:, :, 1])  # HH

                # merged 2MB store, 4KB runs per (partition, band)
                ov = o[:, i0 : i0 + CI].rearrange(
                    "b (s c) (g j) q -> s (c g) b (j q)", c=CI, j=8
                )[0]
                nc.scalar.dma_start(
                    out=ov, in_=ws[:].rearrange("p (b jq) -> p b jq", b=4)
                )

            # NOTE: a free-dim-split variant of the last unit (halved
            # post-last-load chain, measured ~176us) produced NONDETERMINISTIC
            # corruption in the LL/HL bands -- the multi-indexed store-tile
            # views (wv[:, b, j_slice]) appear to lose a DVE->store dependency.
            # Keep the uniform, proven-correct schedule.
            for i0 in range(0, n_images, CI):
                emit_unit(i0)

    nc.compile()
    return nc


_NC_CACHE = {}


def _get_nc(n_images=64):
    if n_images not in _NC_CACHE:
        _NC_CACHE[n_images] = build_nc(n_images)
    return _NC_CACHE[n_images]


def prep_in_maps(x):
    """Host-side input prep: fp16 cast with the Haar /2 folded in (exact),
    plus the even/odd column deinterleave (pure permutation)."""
    x = np.asarray(x)
    assert x.shape == (N_CORES, 64, IMG, IMG), x.shape
    xh = (x * np.float32(0.5)).astype(np.float16)
    # [core, img, g, u, w', eo] -> [core, img, g, eo, u, w']
    xp = np.ascontiguousarray(
        xh.reshape(N_CORES, 64, 32, 16, 256, 2).transpose(0, 1, 2, 5, 3, 4)
    )
    return [{"x": xp[i]} for i in range(N_CORES)]


def kernel(x, **_unused_matrices):
    """Full-input entry point: x [8, 64, 512, 512] f32 -> (LL, LH, HL, HH)."""
    _ensure_concourse()
    from concourse.bass_utils import run_bass_kernel_spmd

    in_maps = prep_in_maps(x)
    nc = _get_nc(64)
    try:
        res = run_bass_kernel_spmd(nc, in_maps, core_ids=list(range(N_CORES)))
    except ImportError:
        # trace=True was forced via BASS_TRACE but this environment lacks the
        # NTFF profiling hook; run untraced instead of failing.
        import os

        os.environ["BASS_NEVER_TRACE"] = "1"
        res = run_bass_kernel_spmd(nc, in_maps, core_ids=list(range(N_CORES)))
    r = res.results
    return tuple(
        np.stack([r[i]["o"][BAND_IDX[b]] for i in range(N_CORES)]).astype(
            np.float32
        )
        for b in BANDS
    )


# revision 27
# speedup vs baseline: 1.0051x; 1.0051x over previous
"""Haar DWT (512x512, levels=1) on 8 Trainium2 NeuronCores.

Input  x: [8, 64, 512, 512] f32  (plus the four Haar band matrices, which
are fixed/deterministic and therefore folded into the kernel math).
Output: (LL, LH, HL, HH), each [8, 64, 256, 256] f32.

Strategy: pure data parallel over the batch dim (core i handles x[i]).
All HBM traffic is fp16 (grading tolerance is 2e-2 rel; fp16 adds ~4e-4)
and the Haar /2 is folded into the host-side cast (x*0.5, exact).

The key layout trick: the host pre-deinterleaves even/odd image COLUMNS
(a pure permutation, folded into the same host-side cast/copy pass that
already exists for the fp16 conversion). With the two column phases
stored as separate contiguous halves, the horizontal butterfly becomes
`even_half +- odd_half` on unit-stride fp16 operands, and the vertical
butterfly pairs adjacent rows within a partition (gappy but unit-stride
inner dim). All six DVE ops per tile therefore run in the 2x perf mode
(2-byte dtype + innermost stride 1 + <=2 free AP dims), unlike the naive
in-order layout whose stride-2 horizontal pass is stuck at 1x. DVE busy
~146us, under the ~152us DMA roofline (64MB/core at the measured
~420GB/s/core aggregate cap), so no PE/ACT assist is needed.

DMA: per unit of 4 images, loads are 4x 512KB dma_starts with 4KB
descriptor runs on the sync queue (4KB is the measured packet sweet
spot; 8KB runs and multi-queue/fine-grained stores all measured slower
end-to-end) and the store is one merged 2MB dma_start on the scalar
queue (bands in one dram tensor, 4KB runs).

Hard-won scheduling facts (each measured as ~+30us when violated):
 - GpSimd must stay COMPLETELY idle -- any op or DMA trigger there
   starves the DMA descriptor path (Q7 cores back it).
 - fio bufs=3 is load-bearing; bufs=2 serializes the pipeline.
 - DVE ops with 3+ free AP dims drop out of 2x mode (1.5ns/elem).
Variants tried and measurably worse: fp8-e4m3 half-input with ACT
upcast (DVE becomes the pacer, +7us), PE-matmul vertical stage for a
subset of images (baseline-style hybrid, +24us), 8KB runs (+16us),
band-interleaved output layout with per-band store chunks (+30us).
"""

import numpy as np


def _ensure_concourse():
    try:
        import concourse.bass  # noqa: F401
    except ImportError:
        import sys

        for p in ("/opt/trn_rl_repo", "/root/.axon_site/_ro/trn_rl_repo"):
            if p not in sys.path:
                sys.path.append(p)
        import concourse.bass  # noqa: F401


N_CORES = 8
IMG = 512  # image height == width
BANDS = ("ll", "lh", "hl", "hh")
# band order inside the merged output tensor
BAND_IDX = {"ll": 0, "lh": 1, "hl": 2, "hh": 3}


def build_nc(n_images=64):
    """Build the single-core Bass program (SPMD: same program on all cores)."""
    _ensure_concourse()
    from concourse import bacc, mybir
    from concourse.tile import TileContext

    f16 = mybir.dt.float16
    # NOTE: keep enable_partition_id at its default (True). Building with
    # False removes a ~3.7 us preamble TENSOR_LOAD but the axon PJRT execute
    # path requires the trailing partition-id parameter and the NEFF faults
    # with NRT_EXEC_UNIT_UNRECOVERABLE without it.
    nc = bacc.Bacc("TRN2", target_bir_lowering=False, debug=False)

    # x layout (host-prepped): [img, g=32, eo=2, u=16, w=256] so that each
    # of the 128 partitions (c g) of a 4-image unit owns 16KB contiguous
    # DRAM: 16 consecutive rows' even-column half then odd-column half.
    x = nc.dram_tensor("x", [n_images, 32, 2, 16, 256], f16,
                       kind="ExternalInput")
    o = nc.dram_tensor("o", [4, n_images, IMG // 2, IMG // 2], f16,
                       kind="ExternalOutput")

    CI = 4          # images per unit
    FX = 2048 * CI  # free elems per partition of the input tile

    with TileContext(nc) as tc:
        with (
            tc.tile_pool(name="fio", bufs=3) as fio_pool,
            tc.tile_pool(name="fmid", bufs=3) as fmid_pool,
            tc.tile_pool(name="fws", bufs=3) as fws_pool,
        ):
            def emit_unit(i0):
                xv = x[i0 : i0 + CI].rearrange("c g eo u w -> (c g) (eo u w)")
                xt = fio_pool.tile([128, FX], f16, tag="x")
                # 4KB descriptor runs (measured best per-packet rate; 16KB
                # packets degrade ~20% under load, 2KB measured 20.5 B/ns
                # vs 4KB's 23-25)
                for k in range(FX // 2048):
                    nc.sync.dma_start(
                        out=xt[:, k * 2048 : (k + 1) * 2048],
                        in_=xv[:, k * 2048 : (k + 1) * 2048],
                    )

                # horizontal butterfly: even half +- odd half, all unit
                # stride -> 2x mode. cs = col sums, cd = col difs.
                xtv = xt[:].rearrange("p (eo m) -> p eo m", eo=2)
                cs = fmid_pool.tile([128, FX // 2], f16, tag="cs")
                cd = fmid_pool.tile([128, FX // 2], f16, tag="cd")
                nc.vector.tensor_add(cs[:], xtv[:, 0], xtv[:, 1])
                nc.vector.tensor_sub(cd[:], xtv[:, 0], xtv[:, 1])

                # vertical butterfly: adjacent row pairs within a partition
                # (inner dim w=256 unit stride -> still 2x mode), written
                # into the four band blocks of one merged store tile.
                ws = fws_pool.tile([128, FX], f16, tag="ws")
                wv = ws[:].rearrange("p (b j w) -> p b j w", b=4, w=256)
                c4 = cs[:].rearrange("p (j eo w) -> p j eo w", eo=2, w=256)
                d4 = cd[:].rearrange("p (j eo w) -> p j eo w", eo=2, w=256)
                nc.vector.tensor_add(wv[:, 0], c4[:, :, 0], c4[:, :, 1])  # LL
                nc.vector.tensor_add(wv[:, 1], d4[:, :, 0], d4[:, :, 1])  # LH
                nc.vector.tensor_sub(wv[:, 2], c4[:, :, 0], c4[:, :, 1])  # HL
                nc.vector.tensor_sub(wv[:, 3], d4[:, :, 0], d4[:, :, 1])  # HH

                # merged 2MB store, 4KB runs per (partition, band)
                ov = o[:, i0 : i0 + CI].rearrange(
                    "b (s c) (g j) q -> s (c g) b (j q)", c=CI, j=8
                )[0]
                nc.scalar.dma_start(
                    out=ov, in_=ws[:].rearrange("p (b jq) -> p b jq", b=4)
                )

            # NOTE: a free-dim-split variant of the last unit (halved
            # post-last-load chain, measured ~176us) produced NONDETERMINISTIC
            # corruption in the LL/HL bands (passed 1 of 2 runs). Suspects:
            # the doubly-indexed op-output views (wv[:, b, j_slice]) and/or
            # the store dma reading PARTIAL regions of ws (wsb[:, :, half])
            # against indexed writes -- every proven store here reads the
            # full tile. Do not split stores or index store tiles twice
            # without independently validating the dependency tracking.
            # Keep the uniform, proven-correct schedule.
            for i0 in range(0, n_images, CI):
                emit_unit(i0)

    nc.compile()
    return nc


_NC_CACHE = {}


def _get_nc(n_images=64):
    if n_images not in _NC_CACHE:
        _NC_CACHE[n_images] = build_nc(n_images)
    return _NC_CACHE[n_images]


def prep_in_maps(x):
    """Host-side input prep: fp16 cast with the Haar /2 folded in (exact),
    plus the even/odd column deinterleave (pure permutation)."""
    x = np.asarray(x)
    assert x.shape == (N_CORES, 64, IMG, IMG), x.shape
    xh = (x * np.float32(0.5)).astype(np.float16)
    # [core, img, g, u, w', eo] -> [core, img, g, eo, u, w']
    xp = np.ascontiguousarray(
        xh.reshape(N_CORES, 64, 32, 16, 256, 2).transpose(0, 1, 2, 5, 3, 4)
    )
    return [{"x": xp[i]} for i in range(N_CORES)]


def kernel(x, **_unused_matrices):
    """Full-input entry point: x [8, 64, 512, 512] f32 -> (LL, LH, HL, HH)."""
    _ensure_concourse()
    from concourse.bass_utils import run_bass_kernel_spmd

    in_maps = prep_in_maps(x)
    nc = _get_nc(64)
    try:
        res = run_bass_kernel_spmd(nc, in_maps, core_ids=list(range(N_CORES)))
    except ImportError:
        # trace=True was forced via BASS_TRACE but this environment lacks the
        # NTFF profiling hook; run untraced instead of failing.
        import os

        os.environ["BASS_NEVER_TRACE"] = "1"
        res = run_bass_kernel_spmd(nc, in_maps, core_ids=list(range(N_CORES)))
    r = res.results
    return tuple(
        np.stack([r[i]["o"][BAND_IDX[b]] for i in range(N_CORES)]).astype(
            np.float32
        )
        for b in BANDS
    )
